# revision 10
# baseline (speedup 1.0000x reference)
"""Cross-attention kernel for Trainium2 (Bass/Tile), 8 NeuronCores.

Computes, per batch b:
    S   = (dom @ ref^T) * SCALE          [N, N]
    P   = softmax(S, axis=-1)
    x   = P @ ref                        [N, C]
    y   = scramble(x)  (x.T flattened and re-chunked into N rows of C)
    out = y @ proj_w^T + proj_b

The scramble + linear fuse algebraically:
    out[2*cp + e, j] = sum_q x[512*e + q, cp] * proj_w[j, q] + proj_b[j]
so out rows with parity e are (x_half_e^T @ proj_w^T) — computed on the
tensor engine with x tiles as lhsT directly; the row interleave (2*cp + e)
is folded into the output DMA access pattern.

Design (v3):
  * All matmul operands bf16 (host-cast): 6.6MB input per core. End-to-end
    rel err vs the fp32 reference ~5.9e-3 (gate is 2e-2). fp8/DoubleRow was
    simulated and rejected: softmax row maxima span too much dynamic range
    for e4m3 (stage2) and even stage3-only fp8 measures 4.8e-2.
  * S is computed TRANSPOSED (S^T = ref @ dom^T): exp(S^T) *is* P^T, which
    stage 2 needs as lhsT — no PE transposes at all. Softmax row-sums come
    from rank-1 ones matmuls ([n,1] PSUM) interleaved with the x matmuls,
    sharing each P^T chunk load; 1/rowsum is applied at x eviction.
  * Every DRAM tensor is host-pre-arranged into its exact SBUF layout
    [128, free] so each DMA is a contiguous-slice copy with 1-4KB elements
    (the v2 fine-grained loads had 256B elements and streamed ~4x slower
    than the PE consumed them).
  * Batch-0-critical loads ride the two HWDGE rings (sync + scalar), which
    start ~1.6us before the SWDGE ring; batch-1 loads go on SWDGE
    (gpsimd). Stores ride sync (idle after the early input triggers).
  * PE warmup matmuls (zeros, no deps) bridge engine-init -> first-input
    so the HAM clock gate is at 8/8 when real work lands.
  * Batches interleave: b0 stage2 -> b1 stage1 -> b0 proj(half1) so the
    PE never waits on the x-evict latency at a half boundary.
"""

import os
from contextlib import ExitStack

import numpy as np
import ml_dtypes

import concourse.bass as bass
import concourse.mybir as mybir
import concourse.tile as tile
from concourse import bacc
from concourse._compat import with_exitstack
from concourse.bass_utils import run_bass_kernel_spmd

B, N, C = 16, 1024, 512
NUM_HEADS = 8
SCALE = (C // NUM_HEADS) ** -0.5  # 0.125
CORES = 8
BPC = B // CORES  # batches per core

P = 128          # partitions
NT = N // P      # 8 query (n) tiles
MT = N // P      # 8 key (m) tiles
KC = C // P      # 4 contraction chunks over channels
MH = 2           # halves of N (PSUM bank = 512 fp32)
JT = C // P      # 4 output-column blocks per half

F32 = mybir.dt.float32
BF16 = mybir.dt.bfloat16

WARMUP_MMS = int(os.environ.get("KERNEL_WARMUP", "10"))


@with_exitstack
def _core_kernel(ctx: ExitStack, tc: tile.TileContext,
                 domt_d, reft_d, ref_d, wt_d, bias_d, out_d):
    nc = tc.nc

    consts = ctx.enter_context(tc.tile_pool(name="consts", bufs=1))

    ps_S = ctx.enter_context(tc.tile_pool(name="ps_s", bufs=3, space="PSUM"))
    ps_X = ctx.enter_context(tc.tile_pool(name="ps_x", bufs=4, space="PSUM"))
    ps_R = ctx.enter_context(tc.tile_pool(name="ps_r", bufs=1, space="PSUM"))

    # PE warmup on memset zeros (gpsimd inits earliest) while inputs stream
    zsrc = consts.tile([P, 640], BF16)
    nc.gpsimd.memset(zsrc[:], 0.0)
    ones_sb = consts.tile([P, 1], BF16)
    nc.gpsimd.memset(ones_sb[:], 1.0)
    if WARMUP_MMS:
        warm_ps = ps_S.tile([P, 512], F32, tag="ps_s")
        for _ in range(WARMUP_MMS):
            nc.tensor.matmul(warm_ps[:], zsrc[:, :P], zsrc[:, P:640],
                             start=True, stop=True)

    p_domT = ctx.enter_context(tc.tile_pool(name="domT", bufs=2))
    p_refT = ctx.enter_context(tc.tile_pool(name="refT", bufs=2))
    p_ref = ctx.enter_context(tc.tile_pool(name="ref", bufs=2))
    p_Pt = ctx.enter_context(tc.tile_pool(name="pt", bufs=2))
    p_x = ctx.enter_context(tc.tile_pool(name="x", bufs=2))
    p_out = ctx.enter_context(tc.tile_pool(name="out", bufs=4))
    p_stats = ctx.enter_context(tc.tile_pool(name="stats", bufs=8))

    # All DRAM tensors are pre-arranged to SBUF layout [128, free]:
    #   domT free = h*2048 + k*512 + j   (rhs slices for S^T)
    #   refT free = mi*512 + k*128 + j   (lhsT slices for S^T)
    #   ref  free = t*512 + c            (rhs chunks for x = P @ ref)
    #   wt   free = q*512 + j            (rhs chunks for the projection)
    domT_sbs = [p_domT.tile([P, MH * KC * 512], BF16, tag="domT",
                            name=f"domT_sb{i}") for i in range(BPC)]
    refT_sbs = [p_refT.tile([P, MT * 512], BF16, tag="refT",
                            name=f"refT_sb{i}") for i in range(BPC)]
    ref_sbs = [p_ref.tile([P, NT * C], BF16, tag="ref", name=f"ref_sb{i}")
               for i in range(BPC)]
    wt_sb = consts.tile([P, KC * C], BF16)
    bias_sb = consts.tile([P, C], F32)

    def span(eng, sb, dr, b, lo, hi):
        eng.dma_start(sb[:, lo:hi], dr[b, :, lo:hi])

    # DMA engines round-robin one descriptor per queue per turn, so a
    # concurrent queue with bigger elements steals bandwidth from the
    # critical stream. ALL inputs therefore ride ONE ring (sync) in strict
    # consumption order; stores ride the scalar ring (disjoint in time).
    span(nc.sync, refT_sbs[0], reft_d, 0, 0, 1024)            # refT b0 pair 0
    span(nc.sync, domT_sbs[0], domt_d, 0, 0, 2048)            # domT b0 h0
    for pp in range(1, MT // 2):                               # refT b0 pairs
        span(nc.sync, refT_sbs[0], reft_d, 0, pp * 1024, (pp + 1) * 1024)
    span(nc.sync, domT_sbs[0], domt_d, 0, 2048, 4096)         # domT b0 h1
    span(nc.sync, ref_sbs[0], ref_d, 0, 0, NT * C)
    nc.sync.dma_start(wt_sb[:], wt_d[:, :])
    nc.sync.dma_start(bias_sb[:], bias_d.partition_broadcast(P))
    if BPC > 1:
        span(nc.sync, domT_sbs[1], domt_d, 1, 0, 4096)
        span(nc.sync, refT_sbs[1], reft_d, 1, 0, 4096)
        span(nc.sync, ref_sbs[1], ref_d, 1, 0, NT * C)

    def stage1(b):
        # S^T = ref @ dom^T (chunked over c); P^T = exp(S^T * SCALE)
        Pt_tiles = [p_Pt.tile([P, N], BF16, tag=f"pt{mi}", name=f"pt{b}_{mi}")
                    for mi in range(MT)]
        for h in range(MH):
            for mi in range(MT):
                ps_s = ps_S.tile([P, 512], F32, tag="ps_s",
                                 name=f"ps_s{b}_{h}_{mi}")
                for k in range(KC):
                    nc.tensor.matmul(
                        ps_s[:],
                        refT_sbs[b][:, mi * 512 + k * P: mi * 512 + (k + 1) * P],
                        domT_sbs[b][:, h * 2048 + k * 512: h * 2048 + (k + 1) * 512],
                        start=(k == 0), stop=(k == KC - 1),
                    )
                nc.scalar.activation(Pt_tiles[mi][:, h * 512:(h + 1) * 512],
                                     ps_s[:],
                                     mybir.ActivationFunctionType.Exp,
                                     scale=float(SCALE))
        return Pt_tiles

    def make_emit_half(b, x_tiles):
        out_v = out_d[b].rearrange("(n2 two) j -> two n2 j", two=2)

        def emit_half_out(e, split_last=False):
            # out rows (2*cp + e) = x_half_e^T @ proj_w^T + bias
            for cb in range(JT):
                ps_z = ps_X.tile([P, C], F32, tag="ps_x",
                                 name=f"ps_z{b}_{e}_{cb}")
                for q in range(KC):
                    x_t = x_tiles[e * KC + q]
                    nc.tensor.matmul(
                        ps_z[:],
                        x_t[:, cb * P:(cb + 1) * P],
                        wt_sb[:, q * C:(q + 1) * C],
                        start=(q == 0), stop=(q == KC - 1),
                    )
                o_sb = p_out.tile([P, C], F32, tag="out", name=f"o{b}_{e}_{cb}")
                # alternate store rings: halves trigger-serialization at the
                # kernel tail and keeps store triggers from delaying the
                # scalar engine's x-evicts during the other batch's stage 2
                ring = nc.scalar if cb % 2 == 0 else nc.sync
                if split_last and cb == JT - 1:
                    # halve the final evict+store so the last bytes leave
                    # ~0.5us sooner
                    for s in range(2):
                        sl = slice(s * 256, (s + 1) * 256)
                        rr = nc.sync if s == 0 else nc.scalar
                        nc.vector.tensor_add(o_sb[:, sl], ps_z[:, sl],
                                             bias_sb[:, sl])
                        rr.dma_start(out_v[e, cb * P:(cb + 1) * P, sl],
                                     o_sb[:, sl])
                else:
                    nc.vector.tensor_add(o_sb[:], ps_z[:], bias_sb[:])
                    ring.dma_start(out_v[e, cb * P:(cb + 1) * P, :], o_sb[:])

        return emit_half_out

    def stage2(b, Pt_tiles, x_tiles, emit_half_out):
        # x = P @ ref, row-sums via rank-1 ones matmuls, normalize at evict
        for nt in range(NT):
            ps_x = ps_X.tile([P, C], F32, tag="ps_x", name=f"ps_x{b}_{nt}")
            ps_r = ps_R.tile([P, 1], F32, tag="ps_r", name=f"ps_r{b}_{nt}")
            for mi in range(MT):
                lhsT = Pt_tiles[mi][:, nt * P:(nt + 1) * P]
                nc.tensor.matmul(ps_x[:], lhsT,
                                 ref_sbs[b][:, mi * C:(mi + 1) * C],
                                 start=(mi == 0), stop=(mi == MT - 1))
                nc.tensor.matmul(ps_r[:], lhsT, ones_sb[:],
                                 start=(mi == 0), stop=(mi == MT - 1))
            recip = p_stats.tile([P, 1], F32, tag="recip", name=f"rc{b}_{nt}")
            nc.vector.reciprocal(recip[:], ps_r[:])
            x_t = p_x.tile([P, C], BF16, tag=f"x{nt}", name=f"x{b}_{nt}")
            nc.scalar.mul(x_t[:], ps_x[:], recip[:])
            x_tiles.append(x_t)
            if nt == KC:
                # half 0's x tiles done one group ago — the gap hides the
                # x-evict latency behind this nt's matmuls
                emit_half_out(0)

    # batch interleave: b0 s1, b0 s2(+proj half0), b1 s1, b0 proj half1,
    # b1 s2(+proj half0), b1 proj half1 — the proj of a finished half
    # always has preceding PE work covering the x-evict latency.
    xs, emits, Pts = {}, {}, {}
    Pts[0] = stage1(0)
    xs[0] = []
    emits[0] = make_emit_half(0, xs[0])
    stage2(0, Pts[0], xs[0], emits[0])
    if BPC > 1:
        Pts[1] = stage1(1)
    emits[0](1)
    if BPC > 1:
        xs[1] = []
        emits[1] = make_emit_half(1, xs[1])
        stage2(1, Pts[1], xs[1], emits[1])
        emits[1](1, split_last=True)


_CACHED = {}


def _build():
    key = ("nc", WARMUP_MMS)
    if key in _CACHED:
        return _CACHED[key]
    nc = bacc.Bacc("TRN2", target_bir_lowering=False, debug=False)
    domt_d = nc.dram_tensor("domt", [BPC, P, MH * KC * 512], BF16,
                            kind="ExternalInput").ap()
    reft_d = nc.dram_tensor("reft", [BPC, P, MT * 512], BF16,
                            kind="ExternalInput").ap()
    ref_d = nc.dram_tensor("ref", [BPC, P, NT * C], BF16,
                           kind="ExternalInput").ap()
    wt_d = nc.dram_tensor("wt", [P, KC * C], BF16, kind="ExternalInput").ap()
    bias_d = nc.dram_tensor("bias", [C], F32, kind="ExternalInput").ap()
    out_d = nc.dram_tensor("out", [BPC, N, C], F32, kind="ExternalOutput").ap()

    with tile.TileContext(nc) as tc:
        _core_kernel(tc, domt_d, reft_d, ref_d, wt_d, bias_d, out_d)
    nc.compile()
    _CACHED[key] = nc
    return nc


LAST_RESULTS = None


def kernel(dom, ref, proj_w, proj_b):
    global LAST_RESULTS
    bf16 = ml_dtypes.bfloat16
    dom = np.asarray(dom, dtype=np.float32)
    ref = np.asarray(ref, dtype=np.float32)
    bias = np.ascontiguousarray(np.asarray(proj_b, dtype=np.float32))

    # SBUF-layout pre-arrangements (free-dim layouts documented in _build)
    domt = np.ascontiguousarray(
        dom.reshape(B, MH, 512, KC, P).transpose(0, 4, 1, 3, 2)
        .reshape(B, P, MH * KC * 512).astype(bf16))
    reft = np.ascontiguousarray(
        ref.reshape(B, MT, P, KC, P).transpose(0, 4, 1, 3, 2)
        .reshape(B, P, MT * 512).astype(bf16))
    refn = np.ascontiguousarray(
        ref.reshape(B, NT, P, C).transpose(0, 2, 1, 3)
        .reshape(B, P, NT * C).astype(bf16))
    wt = np.ascontiguousarray(
        np.asarray(proj_w, dtype=np.float32).T.reshape(KC, P, C)
        .transpose(1, 0, 2).reshape(P, KC * C).astype(bf16))

    nc = _build()
    in_maps = [
        {
            "domt": domt[c * BPC:(c + 1) * BPC],
            "reft": reft[c * BPC:(c + 1) * BPC],
            "ref": refn[c * BPC:(c + 1) * BPC],
            "wt": wt,
            "bias": bias,
        }
        for c in range(CORES)
    ]
    res = run_bass_kernel_spmd(nc, in_maps, list(range(CORES)))
    LAST_RESULTS = res
    if res.exec_time_ns is not None:
        print(f"HW exec time: {res.exec_time_ns} ns")
    return np.concatenate([r["out"] for r in res.results], axis=0)
